# revision 36
# baseline (speedup 1.0000x reference)
"""Trainium2 Bass kernel for nn_Cabasc (aspect-based sentiment model).

Self-contained: takes FULL inputs as numpy arrays, shards batch across 8
NeuronCores (16 seqs/core), runs one SPMD Bass/Tile program, gathers output.

Device program per core (v2: gate-major GRU):
  - GRU recurrence kept entirely in transposed (gate-major) layout: state
    hT96[128, 96] with cols = (dir, h-chunk, lane) and a constant 1.0 row
    for the bhh bias. Weight-stationary matmuls produce psum[gate, lane]
    directly, so there is no per-step transpose or psum->sbuf h copy.
  - gi pipeline: indirect-gather x_l/x_r embeddings, PE-transpose to
    feat-major, weight-stationary gi matmuls pre-fill the r/z psum groups
    (4 steps ahead); n-gate gi staged to sbuf in bf16.
  - Pad-step masking is folded into the z-gate: a pad-indicator column in
    the embedding table injects +40 into z via Wih, so sigma(-Pz) ~= 0 and
    h carries through pad steps with no valid-mask multiply.
  - Per-step attention dot via 6 N=1 matmuls into a psum block, copied to
    sbuf every 16 steps.
  - post phase: unchanged from v1 (attention combine, per-seq kx/score/
    softmax, weighted sums, output MLP + softmax).
"""
import numpy as np
import ml_dtypes

B, L, LA, V, D, H, NP = 128, 512, 5, 32000, 300, 300, 3
NCORES = 8
SPB = 16          # sequences per core
DP = 384          # padded embedding dim (col 300 = bias, col 301 = pad flag)
G3 = 3 * H        # 900

_CACHE = {}
_DBG = {}


def _build(T, debug=False):
    import contextlib
    import concourse.bass as bass
    import concourse.bacc as bacc
    import concourse.tile as tile
    from concourse import mybir

    bf16 = mybir.dt.bfloat16
    f32 = mybir.dt.float32
    i32 = mybir.dt.int32
    AF = mybir.ActivationFunctionType
    OP = mybir.AluOpType
    AX = mybir.AxisListType

    NG4 = (T + 3) // 4          # one gather / gi group covers 4 steps

    nc = bacc.Bacc("TRN2", target_bir_lowering=False, debug=False)

    # ---------------- DRAM inputs ----------------
    embp = nc.dram_tensor("embp", [V, DP], bf16, kind="ExternalInput")
    # weight-stationary GRU weights: [p(128), dir(2), gate(3), gc(3), k(3), m(128)]
    whhT = nc.dram_tensor("whhT", [128, 2, 3, 3, 3, 128], bf16, kind="ExternalInput")
    wihT = nc.dram_tensor("wihT", [128, 2, 3, 3, 3, 128], bf16, kind="ExternalInput")
    wlrD = nc.dram_tensor("wlrD", [128, 2, 3], bf16, kind="ExternalInput")
    wkT = nc.dram_tensor("wkT", [DP, DP], bf16, kind="ExternalInput")
    wprojT = nc.dram_tensor("wprojT", [DP, DP], bf16, kind="ExternalInput")
    wmT = nc.dram_tensor("wmT", [DP, DP], bf16, kind="ExternalInput")
    wdT = nc.dram_tensor("wdT", [DP, 4], bf16, kind="ExternalInput")
    wa1 = nc.dram_tensor("wa1", [DP, 1], bf16, kind="ExternalInput")
    wkwa1 = nc.dram_tensor("wkwa1", [DP, 1], bf16, kind="ExternalInput")
    wkwa1bD = nc.dram_tensor("wkwa1bD", [128, DP], bf16, kind="ExternalInput")
    hv = nc.dram_tensor("hv", [DP, 1], bf16, kind="ExternalInput")
    identD = nc.dram_tensor("identD", [128, 128], bf16, kind="ExternalInput")
    bvecsD = nc.dram_tensor("bvecsD", [DP, 3], f32, kind="ExternalInput")
    bdD = nc.dram_tensor("bdD", [4, 1], f32, kind="ExternalInput")
    sigbD = nc.dram_tensor("sigbD", [64, 1], f32, kind="ExternalInput")
    rmlD = nc.dram_tensor("rmlD", [SPB, 1], f32, kind="ExternalInput")
    ralD = nc.dram_tensor("ralD", [SPB, 1], f32, kind="ExternalInput")
    qacD = nc.dram_tensor("qacD", [SPB, 1], f32, kind="ExternalInput")
    masksD = nc.dram_tensor("masksD", [SPB, 3, L], bf16, kind="ExternalInput")
    gidxD = nc.dram_tensor("gidxD", [128, NG4], i32, kind="ExternalInput")
    h0D = nc.dram_tensor("h0D", [128, 96], bf16, kind="ExternalInput")
    midxD = nc.dram_tensor("midxD", [128, 64], i32, kind="ExternalInput")
    aidxD = nc.dram_tensor("aidxD", [128, 1], i32, kind="ExternalInput")
    smatD = nc.dram_tensor("smatD", [SPB, 128, 4, L], bf16, kind="ExternalInput")

    out_probs = nc.dram_tensor("out_probs", [SPB, 3], f32, kind="ExternalOutput")
    if debug:
        dbg_attn = nc.dram_tensor("dbg_attn", [64, T], f32, kind="ExternalOutput")
        dbg_h = nc.dram_tensor("dbg_h", [128, 96], f32, kind="ExternalOutput")
        dbg_s0 = nc.dram_tensor("dbg_s0", [SPB, L], f32, kind="ExternalOutput")
        dbg_prob = nc.dram_tensor("dbg_prob", [SPB, L], f32, kind="ExternalOutput")
        dbg_scb = nc.dram_tensor("dbg_scb", [SPB, 1], f32, kind="ExternalOutput")
        dbg_wt = nc.dram_tensor("dbg_wt", [SPB, L], f32, kind="ExternalOutput")
        dbg_vs = nc.dram_tensor("dbg_vs", [128, 48], f32, kind="ExternalOutput")
        dbg_uts = nc.dram_tensor("dbg_uts", [128, 48], f32, kind="ExternalOutput")
        dbg_wsh = nc.dram_tensor("dbg_wsh", [SPB, L], f32, kind="ExternalOutput")
        dbg_af = nc.dram_tensor("dbg_af", [64, L], f32, kind="ExternalOutput")
        dbg_arT = nc.dram_tensor("dbg_arT", [128, 4, SPB], f32, kind="ExternalOutput")

    EKT = [(0, 128), (128, 128), (256, 128)]  # embedding k-tiles (384 rows)

    with tile.TileContext(nc) as tc:
        outer = contextlib.ExitStack()
        with outer:
            con = outer.enter_context(tc.tile_pool(name="con", bufs=1))
            sb = outer.enter_context(tc.tile_pool(name="sb", bufs=1))

            # ---------------- constants ----------------
            # gather indices first so the gi pipeline starts immediately,
            # then the GRU weights (two large DMAs), then the rest
            gidx_t = con.tile([128, NG4], i32)
            nc.sync.dma_start(gidx_t[:], gidxD[:])
            midx_t = con.tile([128, 64], i32)
            nc.sync.dma_start(midx_t[:], midxD[:])
            ident = con.tile([128, 128], bf16)
            nc.sync.dma_start(ident[:], identD[:])
            wihT_t = con.tile([128, 2, 3, 3, 3, 128], bf16)
            nc.sync.dma_start(wihT_t[:], wihT[:])
            whhT_t = con.tile([128, 2, 3, 3, 3, 128], bf16)
            nc.sync.dma_start(whhT_t[:], whhT[:])
            wlr_t = con.tile([128, 2, 3], bf16)
            nc.sync.dma_start(wlr_t[:], wlrD[:])
            # post-phase constants
            sigb_t = con.tile([64, 1], f32)
            nc.sync.dma_start(sigb_t[:], sigbD[:])
            aidx_t = con.tile([128, 1], i32)
            nc.sync.dma_start(aidx_t[:], aidxD[:])
            masks_t = con.tile([SPB, 3, L], bf16)
            nc.sync.dma_start(masks_t[:], masksD[:])
            wkT_t = [con.tile([128, DP], bf16, name=f"wkT{k}") for k in range(3)]
            wprojT_t = [con.tile([128, DP], bf16, name=f"wprojT{k}") for k in range(3)]
            wmT_t = [con.tile([128, DP], bf16, name=f"wmT{k}") for k in range(3)]
            wdT_t = [con.tile([128, 4], bf16, name=f"wdT{k}") for k in range(3)]
            wa1_t = [con.tile([128, 1], bf16, name=f"wa1{k}") for k in range(3)]
            wkwa1_t = [con.tile([128, 1], bf16, name=f"wkwa1{k}") for k in range(3)]
            hv_t = [con.tile([128, 1], bf16, name=f"hv{k}") for k in range(3)]
            bvec_t = [con.tile([128, 3], f32, name=f"bvec{k}") for k in range(3)]
            for k, (o, n) in enumerate(EKT):
                nc.sync.dma_start(wkT_t[k][:], wkT[o:o + n, :])
                nc.sync.dma_start(wprojT_t[k][:], wprojT[o:o + n, :])
                nc.sync.dma_start(wmT_t[k][:], wmT[o:o + n, :])
                nc.sync.dma_start(wdT_t[k][:], wdT[o:o + n, :])
                nc.sync.dma_start(wa1_t[k][:], wa1[o:o + n, :])
                nc.sync.dma_start(wkwa1_t[k][:], wkwa1[o:o + n, :])
                nc.sync.dma_start(hv_t[k][:], hv[o:o + n, :])
                nc.sync.dma_start(bvec_t[k][:], bvecsD[o:o + n, :])
            bd_t = con.tile([4, 1], f32)
            nc.sync.dma_start(bd_t[:], bdD[:])
            rml_t = con.tile([SPB, 1], f32)
            nc.sync.dma_start(rml_t[:], rmlD[:])
            ral_t = con.tile([SPB, 1], f32)
            nc.sync.dma_start(ral_t[:], ralD[:])
            qac_t = con.tile([SPB, 1], f32)
            nc.sync.dma_start(qac_t[:], qacD[:])

            # ---------------- persistent state ----------------
            # hT96[p, 48*d + 16*c + b] = h_d[128*c + p] for seq lane b.
            # chunk 2: rows 0:44 = h[256:300], row 44 = 1.0 (bhh bias row).
            hT96 = sb.tile([128, 96], bf16)
            nc.sync.dma_start(hT96[:], h0D[:])
            attn_sb = sb.tile([64, T], f32)
            # post-prep state computed during the GRU phase:
            # vbc = wkwa1 broadcast to all partitions; s0T accumulates the
            # per-token score dots via STT as memory rows arrive.
            vbc = sb.tile([128, DP], bf16)
            nc.sync.dma_start(vbc[:], wkwa1bD[:])
            s0T = sb.tile([128, 4, SPB], f32)
            dump0 = sb.tile([128, DP], bf16)
            mrow_t = [sb.tile([128, 4, DP], bf16, name=f"mrow{b}") for b in range(SPB)]
            smat_t = [sb.tile([128, 4, L], bf16, name=f"smat{b}") for b in range(SPB)]

            # =============== GRU phase ===============
            gru = contextlib.ExitStack()
            with gru:
                work = gru.enter_context(tc.tile_pool(name="work", bufs=2))
                gnp = gru.enter_context(tc.tile_pool(name="gnp", bufs=2))
                emp = gru.enter_context(tc.tile_pool(name="emp", bufs=2))
                ps_r = gru.enter_context(tc.tile_pool(name="ps_r", bufs=2, space="PSUM"))
                ps_z = gru.enter_context(tc.tile_pool(name="ps_z", bufs=2, space="PSUM"))
                ps_gn = gru.enter_context(tc.tile_pool(name="ps_gn", bufs=1, space="PSUM"))
                ps_n = gru.enter_context(tc.tile_pool(name="ps_n", bufs=1, space="PSUM"))
                ps_tr = gru.enter_context(tc.tile_pool(name="ps_tr", bufs=1, space="PSUM"))
                ps_pa = gru.enter_context(tc.tile_pool(name="ps_pa", bufs=1, space="PSUM"))

                Pr_t = [None, None]   # psum [128, 4, 96] per in-flight group
                Pz_t = [None, None]
                gn_t = [None, None]   # sbuf bf16 [128, 4, 96] gi_n staging

                def gather_grp(g):
                    rows = work.tile([128, DP], bf16, name="gthr", tag="gthr")
                    nc.gpsimd.indirect_dma_start(
                        out=rows[:], out_offset=None, in_=embp[:],
                        in_offset=bass.IndirectOffsetOnAxis(ap=gidx_t[:, g:g + 1], axis=0))
                    return rows

                def gi_grp(g, rows):
                    """Transpose gathered rows, run gi matmuls into psum groups."""
                    ptr = ps_tr.tile([128, 3, 128], bf16, name="ptr", tag="ptr")
                    emT = emp.tile([128, 3, 2, 4, 16], bf16, name="emT", tag="emT")
                    for k in range(3):
                        nc.tensor.transpose(ptr[:, k, :], rows[:, 128 * k:128 * (k + 1)],
                                            ident[:])
                        nc.scalar.activation(emT[:, k], ptr[:, k, :], AF.Copy)
                    Pr = ps_r.tile([128, 4, 96], f32, name="Pr", tag="Pr")
                    Pz = ps_z.tile([128, 4, 96], f32, name="Pz", tag="Pz")
                    Gn = ps_gn.tile([128, 4, 96], f32, name="Gn", tag="Gn")
                    gn = gnp.tile([128, 4, 96], bf16, name="gn", tag="gn")
                    for gam, P in ((0, Pr), (1, Pz), (2, Gn)):
                        for d in range(2):
                            for gc in range(3):
                                for s in range(4):
                                    for k in range(3):
                                        nc.tensor.matmul(
                                            P[:, s, 48 * d + 16 * gc:48 * d + 16 * gc + 16],
                                            lhsT=wihT_t[:, d, gam, gc, k, :],
                                            rhs=emT[:, k, d, s, :],
                                            start=(k == 0), stop=(gam == 2 and k == 2),
                                            skip_group_check=True)
                    nc.scalar.activation(gn[:], Gn[:], AF.Copy)
                    Pr_t[g % 2] = Pr
                    Pz_t[g % 2] = Pz
                    gn_t[g % 2] = gn

                def attn_dot(t, pa):
                    j = t % 16
                    for d, lo in ((0, 0), (1, 32)):
                        for c in range(3):
                            nc.tensor.matmul(
                                pa[lo:lo + 16, j:j + 1],
                                lhsT=hT96[:, 48 * d + 16 * c:48 * d + 16 * c + 16],
                                rhs=wlr_t[:, d, c:c + 1],
                                start=(c == 0), stop=(c == 2),
                                skip_group_check=True)

                AHEAD = 2
                pend_rows = {}
                for g in range(min(AHEAD, NG4)):
                    pend_rows[g] = gather_grp(g)
                for g in range(min(AHEAD, NG4)):
                    gi_grp(g, pend_rows.pop(g))
                # schedule of post-prep work interleaved into the step loop:
                # step -> list of (kind, args)
                prep = {}
                for b in range(SPB):
                    for q in range(4):
                        j = 4 * b + q
                        prep.setdefault(min(24 + 2 * j, T - 2), []).append(("gather", b, q))
                        prep.setdefault(min(33 + 2 * j, T - 1), []).append(("stt", b, q))
                for b in range(SPB):
                    prep.setdefault(min(170 + 4 * b, T - 1), []).append(("smat", b))

                pa = ps_pa.tile([64, 16], f32, name="pa", tag="pa")
                for t in range(T):
                    g, s = t // 4, t % 4
                    Pr, Pz, gn = Pr_t[g % 2], Pz_t[g % 2], gn_t[g % 2]
                    # recurrent matmuls: r and z accumulate onto gi psum; n fresh
                    Pn = ps_n.tile([128, 96], f32, name="Pn", tag="Pn")
                    for gam, P in ((0, Pr[:, s, :]), (2, Pn[:]), (1, Pz[:, s, :])):
                        for d in range(2):
                            for gc in range(3):
                                for k in range(3):
                                    nc.tensor.matmul(
                                        P[:, 48 * d + 16 * gc:48 * d + 16 * gc + 16],
                                        lhsT=whhT_t[:, d, gam, gc, k, :],
                                        rhs=hT96[:, 48 * d + 16 * k:48 * d + 16 * k + 16],
                                        start=(gam == 2 and k == 0), stop=(k == 2),
                                        skip_group_check=True)
                    # attention dot for the previous step's h (off the chain)
                    if t > 0:
                        attn_dot(t - 1, pa)
                        if t % 16 == 0:
                            nc.scalar.activation(attn_sb[:, t - 16:t], pa[:], AF.Copy)
                            pa = ps_pa.tile([64, 16], f32, name="pa", tag="pa")
                    # stage Pn to sbuf while the sigmoids run so t1/t2 use
                    # the DVE 2x bf16 mode (GPSIMD cannot touch PSUM)
                    pn_sb = work.tile([128, 96], bf16, name="pn_sb", tag="pn_sb")
                    nc.vector.tensor_copy(pn_sb[:], Pn[:])
                    r_sb = work.tile([128, 96], bf16, name="r_sb", tag="r_sb")
                    nc.scalar.activation(r_sb[:], Pr[:, s, :], AF.Sigmoid)
                    u_sb = work.tile([128, 96], bf16, name="u_sb", tag="u_sb")
                    nc.scalar.activation(u_sb[:], Pz[:, s, :], AF.Sigmoid, scale=-1.0)
                    t1 = work.tile([128, 96], bf16, name="t1", tag="t1")
                    nc.vector.tensor_tensor(out=t1[:], in0=r_sb[:], in1=pn_sb[:], op=OP.mult)
                    t2 = work.tile([128, 96], bf16, name="t2", tag="t2")
                    nc.vector.tensor_tensor(out=t2[:], in0=t1[:], in1=gn[:, s, :], op=OP.add)
                    # off-chain: hg = h - u*h while tanh runs
                    g_sb = work.tile([128, 96], bf16, name="g_sb", tag="g_sb")
                    nc.vector.tensor_tensor(out=g_sb[:], in0=u_sb[:], in1=hT96[:], op=OP.mult)
                    hg_sb = work.tile([128, 96], bf16, name="hg_sb", tag="hg_sb")
                    nc.vector.tensor_tensor(out=hg_sb[:], in0=hT96[:], in1=g_sb[:], op=OP.subtract)
                    n_sb = work.tile([128, 96], bf16, name="n_sb", tag="n_sb")
                    nc.scalar.activation(n_sb[:], t2[:], AF.Tanh)
                    # prefetch / produce gi for future groups (off the h chain)
                    if s == 0:
                        ga = g + AHEAD
                        if ga < NG4:
                            pend_rows[ga] = gather_grp(ga)
                    elif s == 2:
                        ga = g + AHEAD
                        if ga < NG4 and ga in pend_rows:
                            gi_grp(ga, pend_rows.pop(ga))
                    f_sb = work.tile([128, 96], bf16, name="f_sb", tag="f_sb")
                    nc.vector.tensor_tensor(out=f_sb[:], in0=u_sb[:], in1=n_sb[:], op=OP.mult)
                    nc.vector.tensor_tensor(out=hT96[:], in0=hg_sb[:], in1=f_sb[:], op=OP.add)
                    for item in prep.get(t, ()):
                        if item[0] == "gather":
                            b, q = item[1], item[2]
                            nc.gpsimd.indirect_dma_start(
                                out=mrow_t[b][:, q, :], out_offset=None, in_=embp[:],
                                in_offset=bass.IndirectOffsetOnAxis(
                                    ap=midx_t[:, 4 * b + q:4 * b + q + 1], axis=0))
                        elif item[0] == "stt":
                            b, q = item[1], item[2]
                            nc.vector.scalar_tensor_tensor(
                                out=dump0[:], in0=mrow_t[b][:, q, :], scalar=1.0,
                                in1=vbc[:], op0=OP.bypass, op1=OP.mult,
                                accum_out=s0T[:, q, b:b + 1])
                        elif item[0] == "smat":
                            b = item[1]
                            nc.sync.dma_start(smat_t[b][:], smatD[b, :, :, :])
                # final attention dots + tail copy
                attn_dot(T - 1, pa)
                tail = T % 16 if T % 16 else 16
                nc.scalar.activation(attn_sb[:, T - tail:T], pa[:, 16 - tail:16], AF.Copy)

            if debug:
                dbga = sb.tile([64, T], f32)
                nc.vector.tensor_copy(dbga[:], attn_sb[:])
                nc.sync.dma_start(dbg_attn[:], dbga[:])
                dbgh = sb.tile([128, 96], f32)
                nc.vector.tensor_copy(dbgh[:], hT96[:])
                nc.sync.dma_start(dbg_h[:], dbgh[:])

            # =============== post phase ===============
            post = contextlib.ExitStack()
            with post:
                pp = post.enter_context(tc.tile_pool(name="pp", bufs=1))
                pw = post.enter_context(tc.tile_pool(name="pw", bufs=2))
                ps_a = post.enter_context(tc.tile_pool(name="ps_a", bufs=2, space="PSUM"))
                ps_c = post.enter_context(tc.tile_pool(name="ps_c", bufs=2, space="PSUM"))

                # ---- attn sigmoid + 0.5 ----
                af = pp.tile([64, L], bf16)
                nc.vector.memset(af[:], 0.0)
                af_f = pp.tile([64, T], f32)
                nc.scalar.activation(af_f[:], attn_sb[:], AF.Sigmoid, bias=sigb_t[:, 0:1])
                nc.vector.tensor_scalar_add(af[:, 0:T], af_f[:], 0.5)

                # ---- shifted attn_r via per-seq shifted-identity matmul ----
                arT = pp.tile([128, 4, SPB], bf16)
                for q in range(4):
                    pta = ps_a.tile([128, 128], bf16, name="pta", tag="pta")
                    nc.tensor.transpose(pta[0:128, 0:16], af[32:48, 128 * q:128 * (q + 1)],
                                        ident[32:48, 32:48])
                    nc.scalar.activation(arT[:, q, :], pta[0:128, 0:16], AF.Copy)
                wsh = pp.tile([SPB, L], bf16)
                for b in range(SPB):
                    psh = ps_c.tile([1, L], f32, name="psh", tag="ps0")
                    for q in range(4):
                        nc.tensor.matmul(psh[:], lhsT=arT[:, q:q + 1, b:b + 1],
                                         rhs=smat_t[b][:, q, :],
                                         start=(q == 0), stop=(q == 3))
                    shrow = pw.tile([1, L], bf16, name="shrow", tag="shrow")
                    if b % 2 == 0:
                        nc.scalar.activation(shrow[:], psh[:], AF.Copy)
                    else:
                        nc.vector.tensor_copy(shrow[:], psh[:])
                    nc.sync.dma_start(wsh[b:b + 1, :], shrow[:])

                # ---- w combine ----
                w1 = pp.tile([SPB, L], bf16)
                nc.vector.tensor_tensor(out=w1[:], in0=af[0:SPB, :], in1=masks_t[:, 0, :], op=OP.mult)
                w2 = pp.tile([SPB, L], bf16)
                nc.vector.tensor_tensor(out=w2[:], in0=wsh[:], in1=masks_t[:, 1, :], op=OP.mult)
                wt_ = pp.tile([SPB, L], bf16)
                nc.vector.tensor_tensor(out=wt_[:], in0=w1[:], in1=w2[:], op=OP.add)
                nc.vector.tensor_tensor(out=wt_[:], in0=wt_[:], in1=masks_t[:, 2, :], op=OP.add)
                w_n = pp.tile([SPB, L], bf16)
                nc.vector.tensor_scalar_mul(w_n[:], wt_[:], rml_t[:, 0:1])

                # ---- aspect -> score bias (qa + bk.wa1), as [SPB, 1] ----
                arows = pp.tile([128, DP], bf16)
                nc.gpsimd.indirect_dma_start(
                    out=arows[:], out_offset=None, in_=embp[:],
                    in_offset=bass.IndirectOffsetOnAxis(ap=aidx_t[:, 0:1], axis=0))
                aspsum = pp.tile([128, 3, SPB], f32)
                for c in range(3):
                    ptab = ps_a.tile([128, 16, 8], bf16, name="pta2", tag="pta")
                    nc.tensor.transpose(ptab[:], arows[:, 128 * c:128 * (c + 1)], ident[:])
                    nc.vector.tensor_reduce(aspsum[:, c, :], ptab[:, :, 0:5],
                                            axis=AX.X, op=OP.add)
                aspb = pp.tile([128, 3, SPB], bf16)
                nc.vector.tensor_copy(aspb[:], aspsum[:])
                pqa = ps_c.tile([SPB, 1], f32, name="pqa", tag="small")
                for c in range(3):
                    nc.tensor.matmul(pqa[:], lhsT=aspb[:, c, :], rhs=hv_t[c][:, 0:1],
                                     start=(c == 0), stop=(c == 2))
                sc_bias = pp.tile([SPB, 1], f32)
                nc.vector.tensor_tensor(out=sc_bias[:], in0=pqa[:], in1=ral_t[:],
                                        op=OP.mult)
                nc.vector.tensor_scalar_add(sc_bias[:], sc_bias[:], qac_t[:, 0:1])

                # ---- memory pipeline, batched over seqs ----
                # uts[:,b] = Wk @ mv_b + bk,  mv_b = sum_l cv_b[l]*mem_b[l,:]
                # vs[:,b]  = sum_l wn_b[l]*mem_b[l,:]
                # s0raw[b,l] = mem_b[l,:] . (Wk^T wa1)  (bk.wa1 lives in sc_bias)
                # s0T was accumulated during the GRU phase
                # s0T -> row-major s0_all[16, 512] via 4 transposes
                s0Tb = pp.tile([128, 4, SPB], bf16)
                nc.vector.tensor_copy(s0Tb[:], s0T[:])
                ps0r = ps_c.tile([16, 4, 128], bf16, name="ps0r", tag="ps0")
                for q in range(4):
                    nc.tensor.transpose(ps0r[:, q, :], s0Tb[:, q, :], ident[:])
                s0_all = pp.tile([SPB, L], bf16)
                nc.scalar.activation(s0_all[:], ps0r[:], AF.Copy)

                # ---- batched score chain over all seqs ----
                spre = pp.tile([SPB, L], bf16)
                nc.vector.tensor_tensor(out=spre[:], in0=wt_[:], in1=s0_all[:],
                                        op=OP.mult)
                score = pp.tile([SPB, L], f32)
                nc.scalar.activation(score[:], spre[:], AF.Tanh,
                                     bias=sc_bias[:, 0:1])
                ex_t = pp.tile([SPB, L], f32)
                zsum = pp.tile([SPB, 1], f32)
                nc.scalar.activation(ex_t[:], score[:], AF.Exp,
                                     accum_out=zsum[:, 0:1])
                zrec = pp.tile([SPB, 1], f32)
                nc.vector.reciprocal(zrec[:], zsum[:])
                prob = pp.tile([SPB, L], bf16)
                nc.vector.tensor_scalar_mul(prob[:], ex_t[:], zrec[:, 0:1])
                cvr = pp.tile([SPB, L], bf16)
                nc.vector.tensor_tensor(out=cvr[:], in0=prob[:], in1=wt_[:], op=OP.mult)

                # transpose cv/wn into l-major columns [128, (2, 4q, 16b)]
                pcw = ps_a.tile([128, 2, 4, SPB], bf16, name="pcw", tag="pta")
                for q in range(4):
                    nc.tensor.transpose(pcw[:, 0, q, :], cvr[:, 128 * q:128 * (q + 1)],
                                        ident[0:16, 0:16])
                    nc.tensor.transpose(pcw[:, 1, q, :], w_n[:, 128 * q:128 * (q + 1)],
                                        ident[0:16, 0:16])
                cw_sb = pp.tile([128, 2, 4, SPB], bf16)
                nc.scalar.activation(cw_sb[:], pcw[:], AF.Copy)
                # mv (slot 0) and vs (slot 1) per emb chunk c, all seqs
                vsT = pp.tile([128, 3 * SPB], f32)
                utsT = pp.tile([128, 3 * SPB], f32)
                pmv = ps_c.tile([128, 3, 2, SPB], f32, name="pmv", tag="small")
                for c in range(3):
                    for s in range(2):
                        for b in range(SPB):
                            for q in range(4):
                                nc.tensor.matmul(
                                    pmv[:, c, s, b:b + 1],
                                    lhsT=mrow_t[b][:, q, 128 * c:128 * (c + 1)],
                                    rhs=cw_sb[:, s, q, b:b + 1],
                                    start=(q == 0), stop=(q == 3))
                mv_sb = pp.tile([128, 3, SPB], bf16)
                nc.scalar.activation(mv_sb[:], pmv[:, :, 0, :], AF.Copy)
                nc.scalar.activation(vsT[:], pmv[:, :, 1, :], AF.Copy)
                puts = ps_c.tile([128, 3, SPB], f32, name="puts", tag="small")
                for oc in range(3):
                    for b in range(SPB):
                        for ic in range(3):
                            nc.tensor.matmul(
                                puts[:, oc, b:b + 1],
                                lhsT=wkT_t[ic][:, 128 * oc:128 * (oc + 1)],
                                rhs=mv_sb[:, ic, b:b + 1],
                                start=(ic == 0), stop=(ic == 2))
                nc.scalar.activation(utsT[:], puts[:], AF.Copy)
                if debug:
                    for nm_, t_ in (("dbg_s0", None),):
                        pass
                    d1 = pp.tile([SPB, L], f32); nc.vector.tensor_copy(d1[:], s0_all[:]); nc.sync.dma_start(dbg_s0[:], d1[:])
                    d2 = pp.tile([SPB, L], f32); nc.vector.tensor_copy(d2[:], prob[:]); nc.sync.dma_start(dbg_prob[:], d2[:])
                    nc.sync.dma_start(dbg_scb[:], sc_bias[:])
                    d3 = pp.tile([SPB, L], f32); nc.vector.tensor_copy(d3[:], wt_[:]); nc.sync.dma_start(dbg_wt[:], d3[:])
                    d4 = pp.tile([SPB, L], f32); nc.vector.tensor_copy(d4[:], wsh[:]); nc.sync.dma_start(dbg_wsh[:], d4[:])
                    d5 = pp.tile([64, L], f32); nc.vector.tensor_copy(d5[:], af[:]); nc.sync.dma_start(dbg_af[:], d5[:])
                    d6 = pp.tile([128, 4, SPB], f32); nc.vector.tensor_copy(d6[:], arT[:]); nc.sync.dma_start(dbg_arT[:], d6[:])
                    nc.sync.dma_start(dbg_vs[:], vsT[:])
                    nc.sync.dma_start(dbg_uts[:], utsT[:])

                # ---- uts+bk -> Wproj -> +bproj+v_s -> Wm/tanh -> Wd -> softmax ----
                for c in range(3):
                    nc.vector.tensor_scalar_add(utsT[:, c * SPB:(c + 1) * SPB],
                                                utsT[:, c * SPB:(c + 1) * SPB],
                                                bvec_t[c][:, 0:1])
                utsb = pp.tile([128, 3 * SPB], bf16)
                nc.vector.tensor_copy(utsb[:], utsT[:])
                vns = pp.tile([128, 3 * SPB], bf16)
                for oc in range(3):
                    pv = ps_c.tile([128, SPB], f32, name="pv", tag="small")
                    for ic in range(3):
                        nc.tensor.matmul(pv[:],
                                         lhsT=wprojT_t[ic][:, 128 * oc:128 * (oc + 1)],
                                         rhs=utsb[:, ic * SPB:(ic + 1) * SPB],
                                         start=(ic == 0), stop=(ic == 2))
                    nc.vector.scalar_tensor_tensor(
                        out=vns[:, oc * SPB:(oc + 1) * SPB], in0=pv[:],
                        scalar=bvec_t[oc][:, 1:2], in1=vsT[:, oc * SPB:(oc + 1) * SPB],
                        op0=OP.add, op1=OP.add)
                vms = pp.tile([128, 3 * SPB], bf16)
                for oc in range(3):
                    pv2 = ps_c.tile([128, SPB], f32, name="pv2", tag="small")
                    for ic in range(3):
                        nc.tensor.matmul(pv2[:],
                                         lhsT=wmT_t[ic][:, 128 * oc:128 * (oc + 1)],
                                         rhs=vns[:, ic * SPB:(ic + 1) * SPB],
                                         start=(ic == 0), stop=(ic == 2))
                    nc.scalar.activation(vms[:, oc * SPB:(oc + 1) * SPB], pv2[:],
                                         AF.Tanh, bias=bvec_t[oc][:, 2:3])
                plg = ps_c.tile([4, SPB], f32, name="plg", tag="small")
                for ic in range(3):
                    nc.tensor.matmul(plg[:], lhsT=wdT_t[ic][:, 0:4],
                                     rhs=vms[:, ic * SPB:(ic + 1) * SPB],
                                     start=(ic == 0), stop=(ic == 2))
                lgb = pp.tile([4, SPB], bf16)
                nc.vector.tensor_scalar_add(lgb[:], plg[:], bd_t[0:4, 0:1])
                plt = ps_c.tile([SPB, 4], bf16, name="plt", tag="small")
                nc.tensor.matmul(plt[:], lhsT=lgb[:], rhs=ident[0:4, 0:4],
                                 start=True, stop=True, is_transpose=True)
                mneg2 = pp.tile([SPB, 1], f32)
                nc.vector.tensor_reduce(mneg2[:], plt[:, 0:3], axis=AX.X, op=OP.max,
                                        negate=True)
                ex2 = pp.tile([SPB, 3], f32)
                z2 = pp.tile([SPB, 1], f32)
                nc.scalar.activation(ex2[:], plt[:, 0:3], AF.Exp, bias=mneg2[:, 0:1],
                                     accum_out=z2[:, 0:1])
                z2r = pp.tile([SPB, 1], f32)
                nc.vector.reciprocal(z2r[:], z2[:])
                res = pp.tile([SPB, 3], f32)
                nc.vector.tensor_scalar_mul(res[:], ex2[:], z2r[:, 0:1])
                nc.sync.dma_start(out_probs[:], res[:])

    nc.compile()
    return nc


def _h0_init():
    h0 = np.zeros((128, 96), np.float32)
    h0[44, 32:48] = 1.0     # bhh bias row, left half
    h0[44, 80:96] = 1.0     # right half
    return h0


def _host_prep(inputs, T_override=None):
    bf = ml_dtypes.bfloat16
    emb = np.asarray(inputs['embedding'], np.float32)
    ti = np.asarray(inputs['text_raw_indices'])
    ai = np.asarray(inputs['aspect_indices'])
    xl = np.asarray(inputs['x_l'])
    xr = np.asarray(inputs['x_r'])
    mem_len = (ti != 0).sum(-1).astype(np.int64)
    asp_len = (ai != 0).sum(-1).astype(np.int64)
    left_len = (xl != 0).sum(-1).astype(np.int64)
    right_len = (xr != 0).sum(-1).astype(np.int64)
    T = int(max(left_len.max(), right_len.max()))
    if T_override is not None:
        T = T_override

    embp = np.zeros((V, DP), np.float32)
    embp[:, :D] = emb
    embp[:, D] = 1.0
    embp[0, D + 1] = 1.0          # pad-token indicator column
    embp = embp.astype(bf)

    ZBIG = 40.0

    def aug_T(Wg, bg, pad_inject, protect):
        """[128, 3, 3, 3, 128]: (p, gate, gc, k, m) weight-stationary tiles.

        tile[p, gam, gc, k, m] = Wg[300*gam + 128*gc + m, 128*k + p];
        k==2 row 44 = bias bg, row 45 = pad_inject (z only).
        protect: +ZBIG on (z, gc=2, k=2, p=44, m>=44) to hold the ones row.
        """
        Wg = np.asarray(Wg, np.float32)
        bg = np.asarray(bg, np.float32)
        a = np.zeros((128, 3, 3, 3, 128), np.float32)
        for gam in range(3):
            for gc in range(3):
                m_n = 128 if gc < 2 else 44
                rows = slice(300 * gam + 128 * gc, 300 * gam + 128 * gc + m_n)
                for k in range(3):
                    p_n = 128 if k < 2 else 44
                    a[:p_n, gam, gc, k, :m_n] = Wg[rows, 128 * k:128 * k + p_n].T
                a[44, gam, gc, 2, :m_n] = bg[rows]
        if pad_inject:
            a[45, 1, :, 2, :] = ZBIG          # z-gate bump on pad steps
        if protect:
            a[44, 1, 2, 2, 44:] = ZBIG        # keep hT96 ones-row at 1.0
        return a.astype(bf)

    def padT(Wsq):
        a = np.zeros((DP, DP), np.float32)
        a[:D, :D] = np.asarray(Wsq, np.float32).T
        return a.astype(bf)

    # gate order in torch GRU weights: [r, z, n] rows of Wih/Whh
    whhT = np.stack([aug_T(inputs['Whh_l'], inputs['bhh_l'], False, False),
                     aug_T(inputs['Whh_r'], inputs['bhh_r'], False, False)], axis=1)
    wihT = np.stack([aug_T(inputs['Wih_l'], inputs['bih_l'], True, True),
                     aug_T(inputs['Wih_r'], inputs['bih_r'], True, True)], axis=1)

    wlr = np.zeros((128, 2, 3), np.float32)
    for c in range(3):
        n = 128 if c < 2 else 44
        wlr[:n, 0, c] = np.asarray(inputs['wl'], np.float32)[0, 128 * c:128 * c + n]
        wlr[:n, 1, c] = np.asarray(inputs['wr'], np.float32)[0, 128 * c:128 * c + n]

    wa = np.asarray(inputs['w_att'], np.float32)
    wdT = np.zeros((DP, 4), np.float32)
    wdT[:D, :3] = np.asarray(inputs['Wd'], np.float32).T
    bvecs = np.zeros((DP, 3), np.float32)
    bvecs[:D, 0] = np.asarray(inputs['bk'], np.float32)
    bvecs[:D, 1] = np.asarray(inputs['bproj'], np.float32)
    bvecs[:D, 2] = np.asarray(inputs['bm'], np.float32)
    shared = {
        'embp': embp,
        'whhT': whhT.astype(bf),
        'wihT': wihT.astype(bf),
        'wlrD': wlr.astype(bf),
        'wkT': padT(inputs['Wk']),
        'wprojT': padT(inputs['Wproj']),
        'wmT': padT(inputs['Wm']),
        'wdT': wdT.astype(bf),
        'wa1': np.concatenate([wa[:D], np.zeros(DP - D, np.float32)])[:, None].astype(bf),
        'wkwa1': np.concatenate([np.asarray(inputs['Wk'], np.float32).T @ wa[:D],
                                 np.zeros(DP - D, np.float32)])[:, None].astype(bf),
        'wkwa1bD': np.tile(np.concatenate([np.asarray(inputs['Wk'], np.float32).T @ wa[:D],
                                           np.zeros(DP - D, np.float32)])[None, :],
                           (128, 1)).astype(bf),
        'hv': np.concatenate([np.asarray(inputs['Wq'], np.float32).T @ wa[D:],
                              np.zeros(DP - D, np.float32)])[:, None].astype(bf),
        'identD': np.eye(128, dtype=np.float32).astype(bf),
        'h0D': _h0_init().astype(bf),
        'bvecsD': bvecs,
        'bdD': np.concatenate([np.asarray(inputs['bd'], np.float32),
                               [0.0]])[:, None].astype(np.float32),
    }
    qa_c = float(np.asarray(inputs['bq'], np.float32) @ wa[D:]) + \
        float(np.asarray(inputs['bk'], np.float32) @ wa[:D])

    NG4 = (T + 3) // 4
    per_core = []
    for c in range(NCORES):
        sl = slice(c * SPB, (c + 1) * SPB)
        xlc, xrc = xl[sl], xr[sl]
        llc = left_len[sl]
        mlc, alc = mem_len[sl], asp_len[sl]
        a_start = (llc - alc).astype(np.int64)

        gidx = np.zeros((128, NG4), np.int32)
        for g in range(NG4):
            for s in range(4):
                t = 4 * g + s
                if t >= T:
                    continue
                gidx[16 * s:16 * s + 16, g] = xlc[:, t]
                gidx[64 + 16 * s:64 + 16 * s + 16, g] = xrc[:, t]
        midx = np.zeros((128, 64), np.int32)
        for b in range(SPB):
            for q in range(4):
                midx[:, 4 * b + q] = ti[sl][b, 128 * q:128 * (q + 1)]
        aidx = np.zeros((128, 1), np.int32)
        for b in range(SPB):
            aidx[8 * b:8 * b + 5, 0] = ai[sl][b, :]

        idxL = np.arange(L)[None, :]
        mL = (idxL < llc[:, None]).astype(np.float32)
        mR = ((idxL >= a_start[:, None]) & (idxL < mlc[:, None])).astype(np.float32)
        mP = (idxL >= mlc[:, None]).astype(np.float32)
        masks = np.stack([mL, mR, mP], axis=1).astype(bf)

        smat = np.zeros((SPB, 128, 4, L), np.float32)
        for b in range(SPB):
            s = int(a_start[b])
            jj = np.arange(L - s)
            smat[b, jj % 128, jj // 128, jj + s] = 1.0
        sig_b = np.zeros((64, 1), np.float32)
        sig_b[0:SPB, 0] = float(np.asarray(inputs['bl'])[0])
        sig_b[32:32 + SPB, 0] = float(np.asarray(inputs['br'])[0])

        pc = dict(shared)
        pc.update({
            'gidxD': gidx, 'midxD': midx, 'aidxD': aidx,
            'masksD': masks,
            'sigbD': sig_b,
            'rmlD': (1.0 / mlc.astype(np.float32))[:, None],
            'ralD': (1.0 / alc.astype(np.float32))[:, None],
            'qacD': np.full((SPB, 1), qa_c, np.float32),
            'smatD': smat.astype(bf),
        })
        per_core.append(pc)
    return T, per_core


def kernel(**inputs):
    from concourse.bass_utils import run_bass_kernel_spmd
    T, per_core = _host_prep(inputs)
    key = ("v2", T)
    if key not in _CACHE:
        _CACHE[key] = _build(T, debug=False)
    nc = _CACHE[key]
    res = run_bass_kernel_spmd(nc, per_core, list(range(NCORES)))
    out = np.zeros((B, NP), np.float32)
    for c in range(NCORES):
        out[c * SPB:(c + 1) * SPB, :] = res.results[c]["out_probs"]
    return out
